# revision 33
# baseline (speedup 1.0000x reference)
"""Trainium2 Bass kernel for nn_Attention_49366354100559.

Multi-head attention: B=2, T=2048, D=768, H=12, Dh=64.
Reference zeroes the upper triangle of scores (not -inf) before softmax,
so masked positions contribute exp(0)=1 to the softmax — the attention
matrix is dense in attn@v.

Sharding: 8 cores = 2 batches x 4 core-groups; each core computes 3 heads
of one batch and produces a partial [2048, 768] output (pre-W_o-bias);
host sums the 4 partials per batch and adds b_o.

Per-core device program (fp16 matmul operands, 1 cycle/row PE):
  1. x^T is pre-transposed and cast to fp16 on the HOST and DMA'd in as
     6 [128, 2048] tiles (column-chunked so projections start early);
     a dozen warmup matmuls un-throttle the PE HAM clock gate during the
     first DMA wait.
  2. q^T,k^T feature-major with W stationary (column groups [q0|q1],
     [k0|k1],[q2|k2] so each head's q/k share a partition base; k2 is
     moved to base 0 with an SBUF-SBUF DMA). v token-major with x^T
     stationary, plus an appended ones column (v_aug) so attn@v also
     accumulates the softmax denominator for free.
  3. Attention as a software pipeline over units (head, k-quad):
     scores^T[k,q] = k @ q^T on live columns only, exp on ACT straight
     out of PSUM; the always-masked band is memset to 1.0 and the
     128-wide causal edge fixed with affine_select(fill=1.0); attn@v of
     the previous unit's exp rows runs between score bursts to keep the
     PE dense (HAM stays warm). Fully-masked k-tiles are replaced by
     per-quad v column-sum suffixes.
  4. Finalize per (head, q-group): fast reciprocal of the denominator
     row, partition-broadcast, scale -> attn_out^T.
  5. O-projection per token tile, interleaved with the last head; y is
     written fp16 (host sums partials in fp32).
"""

import sys

import numpy as np

if "/opt/trn_rl_repo" not in sys.path:
    sys.path.insert(0, "/opt/trn_rl_repo")

import concourse.mybir as mybir
from concourse import bacc
from concourse.tile import TileContext
from concourse.bass_utils import run_bass_kernel_spmd

F32 = mybir.dt.float32
F16 = mybir.dt.float16
AF = mybir.ActivationFunctionType
ALU = mybir.AluOpType

MMDT = F16
NPDT = np.float16

N_CORES = 8
VN = 192
T = 2048
D = 768
HPC = 3  # heads per core
DH = 64
NK = 16  # k-token tiles of 128
NG = 4  # q groups of 512
KT = 6  # contraction tiles for D=768


def build_nc():
    nc = bacc.Bacc("TRN2", target_bir_lowering=False, debug=False,
                   num_devices=N_CORES)
    d = {}
    d["xt"] = nc.dram_tensor("xt", [D, T], MMDT, kind="ExternalInput").ap()
    d["wqk"] = nc.dram_tensor("wqk", [D, 384], MMDT, kind="ExternalInput").ap()
    d["bqk"] = nc.dram_tensor("bqk", [128, 3], F32, kind="ExternalInput").ap()
    d["wv"] = nc.dram_tensor("wv", [D, VN], MMDT, kind="ExternalInput").ap()
    d["bv"] = nc.dram_tensor("bv", [128, VN], F32, kind="ExternalInput").ap()
    d["wo01"] = nc.dram_tensor("wo01", [128, D], MMDT,
                               kind="ExternalInput").ap()
    d["wo2"] = nc.dram_tensor("wo2", [DH, D], MMDT,
                              kind="ExternalInput").ap()
    d["ones"] = nc.dram_tensor("ones", [128, 512], MMDT,
                               kind="ExternalInput").ap()
    d["y"] = nc.dram_tensor("y", [T, D], MMDT, kind="ExternalOutput").ap()

    with TileContext(nc) as tc:
        _emit(nc, tc, d)
    nc.compile()
    return nc


def _emit(nc, tc, d):
    from contextlib import ExitStack

    with ExitStack() as ctx:
        wp = ctx.enter_context(tc.tile_pool(name="wp", bufs=1))
        main = ctx.enter_context(tc.tile_pool(name="main", bufs=1))

        # ---- weight/constant tiles ----
        wqk = [wp.tile([128, 384], MMDT, tag=f"wqk{k}", name=f"wqk{k}")
               for k in range(KT)]
        wv = [wp.tile([128, VN], MMDT, tag=f"wv{k}", name=f"wv{k}")
              for k in range(KT)]
        wo01 = wp.tile([128, D], MMDT, tag="wo01", name="wo01")
        wo2 = wp.tile([DH, D], MMDT, tag="wo2", name="wo2")
        bqk = wp.tile([128, 3], F32, tag="bqk", name="bqk")
        bv = wp.tile([128, VN], F32, tag="bv", name="bv")
        ones = wp.tile([128, 512], MMDT, tag="ones", name="ones")

        # ---- persistent SBUF ----
        qkt = [main.tile([128, T], MMDT, tag=f"qkt{g}", name=f"qkt{g}")
               for g in range(3)]  # [q0|q1], [k0|k1], [q2|k2]
        alt2 = main.tile([128, T], MMDT, tag="alt2", name="alt2")
        vaug = main.tile([128, HPC * NK * 65], MMDT, tag="vaug", name="vaug")
        aout = {h: main.tile([DH, T], MMDT, tag=f"aout{h}", name=f"aout{h}")
                for h in (1, 2)}
        aout01 = main.tile([128, T], MMDT, tag="aout01", name="aout01")
        accs = [main.tile([65, 512], F32, tag=f"acc{g}", name=f"acc{g}")
                for g in range(NG)]
        vsum = [[main.tile([128, 65], MMDT, tag=f"vs{h}{g}",
                           name=f"vs{h}{g}") for g in range(3)]
                for h in range(HPC)]

        # ============ phase 1: DMA in (xT pre-transposed on host) ========
        xT_ctx = ExitStack()
        xTp = xT_ctx.enter_context(tc.tile_pool(name="xTp", bufs=1))
        xT = [xTp.tile([128, T], MMDT, tag=f"xT{f}", name=f"xT{f}")
              for f in range(KT)]

        nc.scalar.dma_start(bqk[:], d["bqk"])
        for k in range(KT):
            nc.scalar.dma_start(wqk[k][:], d["wqk"][k * 128:(k + 1) * 128, :])
        nc.scalar.dma_start(ones[:], d["ones"])
        # first column-chunk of xT in fine 256-col pieces across queues
        for k in range(KT):
            for half in range(2):
                c0 = 256 * half
                nc.sync.dma_start(
                    xT[k][:, c0:c0 + 256],
                    d["xt"][k * 128:(k + 1) * 128, c0:c0 + 256])
        for c in range(1, NG):
            for k in range(KT):
                nc.sync.dma_start(
                    xT[k][:, c * 512:(c + 1) * 512],
                    d["xt"][k * 128:(k + 1) * 128, c * 512:(c + 1) * 512])
        for k in range(KT):
            nc.scalar.dma_start(wv[k][:], d["wv"][k * 128:(k + 1) * 128, :])
        nc.scalar.dma_start(bv[:], d["bv"])
        nc.scalar.dma_start(wo01[:], d["wo01"])
        nc.scalar.dma_start(wo2[:], d["wo2"])

        # vaug softmax-denominator ones columns
        va4 = vaug.rearrange("p (h k c) -> p h k c", h=HPC, c=65)
        for h in range(HPC):
            nc.vector.tensor_copy(
                va4[:, h, :, 64], ones[:, 0:1].broadcast_to([128, NK]))

        # ============ phase 2: warmup + projections ============
        vau3 = vaug.rearrange("p (h x) -> p h x", h=HPC)
        with tc.tile_pool(name="pps", bufs=2, space="PSUM") as pps:
            wu = pps.tile([3, 3], F32, tag="wu", name="wu", bufs=1)
            for i in range(45):
                nc.tensor.matmul(wu[:], bqk[:], bqk[:],
                                 start=True, stop=True,
                                 skip_group_check=True)
            for n in range(NG):
                for g in range(3):
                    ps = pps.tile([128, 512], F32, tag="qk", name=f"qk{g}_{n}")
                    for k in range(KT):
                        nc.tensor.matmul(
                            ps[:], wqk[k][:, g * 128:(g + 1) * 128],
                            xT[k][:, n * 512:(n + 1) * 512],
                            start=(k == 0), stop=(k == KT - 1))
                    nc.vector.tensor_scalar_add(
                        qkt[g][:, n * 512:(n + 1) * 512], ps[:],
                        bqk[:, g:g + 1])
                for tt in range(4 * n, min(4 * n + 4, 12)):
                    ps = pps.tile([128, VN], F32, tag="v", name=f"v{tt}")
                    for k in range(KT):
                        nc.tensor.matmul(
                            ps[:], xT[k][:, tt * 128:(tt + 1) * 128], wv[k][:],
                            start=(k == 0), stop=(k == KT - 1))
                    nc.vector.tensor_add(
                        vau3[:, :, tt * 65:tt * 65 + 64],
                        ps.rearrange("p (h c) -> p h c", h=HPC),
                        bv.rearrange("p (h c) -> p h c", h=HPC))
            # alt2 = T3 with halves swapped (partition-shifting DMAs), so
            # h2's consecutive k-tiles can use alternating row groups
            nc.sync.dma_start(alt2[0:64, :], qkt[2][64:128, :])
            nc.sync.dma_start(alt2[64:128, :], qkt[2][0:64, :])

        def vsums():
            # masked-tile V sums: vsum[h][g] = sum_{ki >= 4(g+1)} vaug_ki,
            # consumed by one extra attn@v matmul against the ones tile
            with nc.allow_low_precision(reason="sum of <=12 fp16 values"):
                for h in range(HPC):
                    va3 = vaug[:, 1040 * h:1040 * (h + 1)].rearrange(
                        "p (k c) -> p c k", c=65)
                    for g in range(3):
                        nc.vector.tensor_reduce(
                            vsum[h][g][:], va3[:, :, 4 * (g + 1):NK],
                            axis=mybir.AxisListType.X, op=ALU.add)

        # ============ phase 3+4: attention pipeline + O-projection ======
        headqk = [
            (qkt[0][0:64, :], qkt[1][0:64, :]),      # h0: base 0
            (qkt[0][64:128, :], qkt[1][64:128, :]),  # h1: base 64
            (qkt[2][0:64, :], alt2[0:64, :]),        # h2: base 0
        ]
        fill1 = nc.gpsimd.to_reg(1.0)

        with tc.tile_pool(name="ep", bufs=12) as ep, \
             tc.tile_pool(name="fin", bufs=2) as fin, \
             tc.tile_pool(name="outp", bufs=3) as outp, \
             tc.tile_pool(name="sps", bufs=2, space="PSUM") as sps, \
             tc.tile_pool(name="ops", bufs=2, space="PSUM") as ops, \
             tc.tile_pool(name="oprj", bufs=2, space="PSUM") as oprj:

            erows = {}

            def unit_score_steps(u):
                h, J = u
                qT, kT = headqk[h]
                steps = []
                for j in range(4):
                    ki = 4 * J + j

                    def step(ki=ki):
                        lo = 128 * ki
                        e = ep.tile([128, T], MMDT, tag="e", name=f"e{h}_{ki}")
                        erows[(h, ki)] = e
                        for P in range(lo // 1024, 2):
                            clo = max(lo, 1024 * P)
                            ps = sps.tile([128, 1024], F32, tag="s",
                                          name=f"s{h}_{ki}_{P}")
                            for n in range(2):
                                s0 = 1024 * P + 512 * n
                                if s0 + 512 <= lo:
                                    continue
                                a0 = max(s0, lo)  # trim masked band columns
                                nc.tensor.matmul(
                                    ps[:, a0 - 1024 * P:512 * (n + 1)],
                                    kT[:, lo:lo + 128], qT[:, a0:s0 + 512])
                            nc.scalar.activation(
                                e[:, clo:1024 * (P + 1)],
                                ps[:, clo - 1024 * P:1024], AF.Exp,
                                scale=0.125)
                        w0 = 512 * J  # start of this unit's read window
                        if lo > w0:
                            # always-masked band: exp(0) = 1
                            nc.gpsimd.memset(e[:, w0:lo], 1.0)
                        nc.gpsimd.affine_select(
                            e[:, lo:lo + 128], e[:, lo:lo + 128],
                            pattern=[[1, 128]], compare_op=ALU.is_ge,
                            fill=fill1, base=0,
                            channel_multiplier=-1)
                    steps.append(step)
                return steps

            def unit_attnv_groups(u):
                h, J = u
                groups = []
                for g in range(J, NG):

                    def grp(g=g):
                        po = ops.tile([65, 512], F32, tag="o",
                                      name=f"o{h}{J}{g}")
                        has_virtual = (J == g and g < 3)
                        for j in range(4):
                            ki = 4 * J + j
                            nc.tensor.matmul(
                                po[:],
                                vaug[:, 1040 * h + ki * 65:
                                     1040 * h + ki * 65 + 65],
                                erows[(h, ki)][:, 512 * g:512 * (g + 1)],
                                start=(j == 0),
                                stop=(j == 3 and not has_virtual))
                        if has_virtual:
                            # masked k-tiles: weight-1 contribution of the
                            # precomputed V suffix sums
                            nc.tensor.matmul(po[:], vsum[h][g][:], ones[:],
                                             start=False, stop=True)
                        if J == 0:
                            nc.vector.tensor_copy(accs[g][:], po[:])
                        else:
                            nc.vector.tensor_add(accs[g][:], po[:], accs[g][:])
                        if J == g:
                            den = fin.tile([1, 512], F32, tag="den",
                                           name=f"den{h}{g}")
                            scr = fin.tile([1, 512], F32, tag="scr",
                                           name=f"scr{h}{g}")
                            rb = fin.tile([DH, 512], F32, tag="rb",
                                          name=f"rb{h}{g}")
                            nc.vector.tensor_copy(den[:], accs[g][64:65, :])
                            nc.vector.reciprocal_approx_fast(scr[:], den[:])
                            nc.gpsimd.partition_broadcast(rb[:], scr[:])
                            if h == 0:
                                dst = aout01[0:64, 512 * g:512 * (g + 1)]
                            else:
                                dst = aout[h][:, 512 * g:512 * (g + 1)]
                            nc.vector.tensor_mul(
                                dst, accs[g][0:64, :], rb[:])
                            if h == 1:
                                # stack h1 under h0 (partition-shift DMA)
                                nc.sync.dma_start(
                                    aout01[64:128, 512 * g:512 * (g + 1)],
                                    aout[1][:, 512 * g:512 * (g + 1)])
                            if h == 2:
                                oproj_group(g)
                    groups.append(grp)
                return groups

            def oproj_group(tg):
                for tt in range(4 * tg, 4 * tg + 4):
                    ot = outp.tile([128, D], MMDT, tag="ot", name=f"ot{tt}")
                    for (n0, w) in ((0, 512), (512, 256)):
                        ps = oprj.tile([128, 512], F32, tag="op",
                                       name=f"op{tt}_{n0}")
                        nc.tensor.matmul(
                            ps[:, 0:w],
                            aout01[:, tt * 128:(tt + 1) * 128],
                            wo01[:, n0:n0 + w], start=True, stop=False)
                        nc.tensor.matmul(
                            ps[:, 0:w],
                            aout[2][:, tt * 128:(tt + 1) * 128],
                            wo2[:, n0:n0 + w], start=False, stop=True)
                        nc.vector.tensor_copy(ot[:, n0:n0 + w], ps[:, 0:w])
                    r = slice(tt * 128, (tt + 1) * 128)
                    w = D // (4 if tg == 3 else 2)
                    for p0 in range(0, D, w):
                        nc.sync.dma_start(d["y"][r, p0:p0 + w],
                                          ot[:, p0:p0 + w])

            def v_group_s(tt, tag):
                ps = sps.tile([128, VN], F32, tag="s", name=f"v{tt}")
                for k in range(KT):
                    nc.tensor.matmul(
                        ps[:], xT[k][:, tt * 128:(tt + 1) * 128], wv[k][:],
                        start=(k == 0), stop=(k == KT - 1))
                nc.vector.tensor_add(
                    vau3[:, :, tt * 65:tt * 65 + 64],
                    ps.rearrange("p (h c) -> p h c", h=HPC),
                    bv.rearrange("p (h c) -> p h c", h=HPC))

            units = [(h, J) for h in range(HPC) for J in range(4)]
            pending = [lambda tt=tt: v_group_s(tt, "s") for tt in range(12, 16)]
            pending.append(vsums)
            for u in units:
                steps = unit_score_steps(u)
                per = (len(pending) + len(steps) - 1) // max(len(steps), 1)
                gi = 0
                for st in steps:
                    st()
                    for _ in range(per):
                        if gi < len(pending):
                            pending[gi]()
                            gi += 1
                while gi < len(pending):
                    pending[gi]()
                    gi += 1
                pending = unit_attnv_groups(u)
            for grp in pending:
                grp()

        xT_ctx.close()


_NC_CACHE = None


def _get_nc():
    global _NC_CACHE
    if _NC_CACHE is None:
        _NC_CACHE = build_nc()
    return _NC_CACHE


def _make_in_maps(residual_stream, W_q, b_q, W_k, b_k, W_v, b_v, W_o, b_o):
    xts = [np.ascontiguousarray(residual_stream[b].T).astype(NPDT)
           for b in range(residual_stream.shape[0])]
    in_maps = []
    for c in range(N_CORES):
        b = c // 4
        hs = [3 * (c % 4) + i for i in range(HPC)]
        cs = [slice(64 * h, 64 * h + 64) for h in hs]
        wqk = np.concatenate(
            [W_q[:, cs[0]], W_q[:, cs[1]], W_k[:, cs[0]], W_k[:, cs[1]],
             W_q[:, cs[2]], W_k[:, cs[2]]], axis=1).astype(NPDT)
        bqk = np.concatenate(
            [b_q[cs[0]], b_q[cs[1]], b_k[cs[0]], b_k[cs[1]],
             b_q[cs[2]], b_k[cs[2]]]).astype(np.float32)
        bqk = np.ascontiguousarray(bqk.reshape(3, 128).T)
        wv = np.ascontiguousarray(
            np.concatenate([W_v[:, s] for s in cs], axis=1)).astype(NPDT)
        bv = np.zeros((1, VN), dtype=np.float32)
        bv[0, :] = np.concatenate([b_v[s] for s in cs])
        bv = np.ascontiguousarray(np.broadcast_to(bv, (128, VN)))
        m = {
            "xt": xts[b],
            "wqk": wqk,
            "bqk": bqk,
            "wv": wv,
            "bv": bv,
            "ones": np.ones((128, 512), dtype=NPDT),
        }
        m["wo01"] = np.ascontiguousarray(
            W_o[64 * hs[0]:64 * hs[0] + 128, :]).astype(NPDT)
        m["wo2"] = np.ascontiguousarray(
            W_o[64 * hs[2]:64 * hs[2] + 64, :]).astype(NPDT)
        in_maps.append(m)
    return in_maps


def kernel(residual_stream, W_q, b_q, W_k, b_k, W_v, b_v, W_o, b_o,
           _trace=False):
    residual_stream = np.asarray(residual_stream, dtype=np.float32)
    args = [np.asarray(a, dtype=np.float32)
            for a in (W_q, b_q, W_k, b_k, W_v, b_v, W_o, b_o)]
    W_q, b_q, W_k, b_k, W_v, b_v, W_o, b_o = args
    nc = _get_nc()
    in_maps = _make_in_maps(residual_stream, W_q, b_q, W_k, b_k, W_v, b_v,
                            W_o, b_o)
    res = run_bass_kernel_spmd(nc, in_maps, core_ids=list(range(N_CORES)),
                               trace=_trace)
    B = residual_stream.shape[0]
    out = np.zeros((B, T, D), dtype=np.float32)
    for c in range(N_CORES):
        out[c // 4] += res.results[c]["y"].astype(np.float32)
    out += b_o[None, None, :]
    if _trace:
        kernel._last_result = res
    return out


# revision 34
# speedup vs baseline: 1.0098x; 1.0098x over previous
"""Trainium2 Bass kernel for nn_Attention_49366354100559.

Multi-head attention: B=2, T=2048, D=768, H=12, Dh=64.
Reference zeroes the upper triangle of scores (not -inf) before softmax,
so masked positions contribute exp(0)=1 to the softmax — the attention
matrix is dense in attn@v.

Sharding: 8 cores = 2 batches x 4 core-groups; each core computes 3 heads
of one batch and produces a partial [2048, 768] output (pre-W_o-bias);
host sums the 4 partials per batch and adds b_o.

Per-core device program (fp16 matmul operands, 1 cycle/row PE):
  1. x^T is pre-transposed and cast to fp16 on the HOST and DMA'd in as
     6 [128, 2048] tiles (column-chunked so projections start early);
     a dozen warmup matmuls un-throttle the PE HAM clock gate during the
     first DMA wait.
  2. q^T,k^T feature-major with W stationary (column groups [q0|q1],
     [k0|k1],[q2|k2] so each head's q/k share a partition base; k2 is
     moved to base 0 with an SBUF-SBUF DMA). v token-major with x^T
     stationary, plus an appended ones column (v_aug) so attn@v also
     accumulates the softmax denominator for free.
  3. Attention as a software pipeline over units (head, k-quad):
     scores^T[k,q] = k @ q^T on live columns only, exp on ACT straight
     out of PSUM; the always-masked band is memset to 1.0 and the
     128-wide causal edge fixed with affine_select(fill=1.0); attn@v of
     the previous unit's exp rows runs between score bursts to keep the
     PE dense (HAM stays warm). Fully-masked k-tiles are replaced by
     per-quad v column-sum suffixes.
  4. Finalize per (head, q-group): fast reciprocal of the denominator
     row, partition-broadcast, scale -> attn_out^T.
  5. O-projection per token tile, interleaved with the last head; y is
     written fp16 (host sums partials in fp32).
"""

import sys

import numpy as np

if "/opt/trn_rl_repo" not in sys.path:
    sys.path.insert(0, "/opt/trn_rl_repo")

import concourse.mybir as mybir
from concourse import bacc
from concourse.tile import TileContext
from concourse.bass_utils import run_bass_kernel_spmd

F32 = mybir.dt.float32
F16 = mybir.dt.float16
AF = mybir.ActivationFunctionType
ALU = mybir.AluOpType

MMDT = F16
NPDT = np.float16

N_CORES = 8
VN = 192
T = 2048
D = 768
HPC = 3  # heads per core
DH = 64
NK = 16  # k-token tiles of 128
NG = 4  # q groups of 512
KT = 6  # contraction tiles for D=768


def build_nc():
    nc = bacc.Bacc("TRN2", target_bir_lowering=False, debug=False,
                   num_devices=N_CORES)
    d = {}
    d["xt"] = nc.dram_tensor("xt", [D, T], MMDT, kind="ExternalInput").ap()
    d["wqk"] = nc.dram_tensor("wqk", [D, 384], MMDT, kind="ExternalInput").ap()
    d["bqk"] = nc.dram_tensor("bqk", [128, 3], F32, kind="ExternalInput").ap()
    d["wv"] = nc.dram_tensor("wv", [D, VN], MMDT, kind="ExternalInput").ap()
    d["bv"] = nc.dram_tensor("bv", [128, VN], F32, kind="ExternalInput").ap()
    d["wo01"] = nc.dram_tensor("wo01", [128, D], MMDT,
                               kind="ExternalInput").ap()
    d["wo2"] = nc.dram_tensor("wo2", [DH, D], MMDT,
                              kind="ExternalInput").ap()
    d["ones"] = nc.dram_tensor("ones", [128, 512], MMDT,
                               kind="ExternalInput").ap()
    d["y"] = nc.dram_tensor("y", [T, D], MMDT, kind="ExternalOutput").ap()

    with TileContext(nc) as tc:
        _emit(nc, tc, d)
    nc.compile()
    return nc


def _emit(nc, tc, d):
    from contextlib import ExitStack

    with ExitStack() as ctx:
        wp = ctx.enter_context(tc.tile_pool(name="wp", bufs=1))
        main = ctx.enter_context(tc.tile_pool(name="main", bufs=1))

        # ---- weight/constant tiles ----
        wqk = [wp.tile([128, 384], MMDT, tag=f"wqk{k}", name=f"wqk{k}")
               for k in range(KT)]
        wv = [wp.tile([128, VN], MMDT, tag=f"wv{k}", name=f"wv{k}")
              for k in range(KT)]
        wo01 = wp.tile([128, D], MMDT, tag="wo01", name="wo01")
        wo2 = wp.tile([DH, D], MMDT, tag="wo2", name="wo2")
        bqk = wp.tile([128, 3], F32, tag="bqk", name="bqk")
        bv = wp.tile([128, VN], F32, tag="bv", name="bv")
        ones = wp.tile([128, 512], MMDT, tag="ones", name="ones")

        # ---- persistent SBUF ----
        qkt = [main.tile([128, T], MMDT, tag=f"qkt{g}", name=f"qkt{g}")
               for g in range(3)]  # [q0|q1], [k0|k1], [q2|k2]
        alt2 = main.tile([128, T], MMDT, tag="alt2", name="alt2")
        vaug = main.tile([128, HPC * NK * 65], MMDT, tag="vaug", name="vaug")
        aout = {h: main.tile([DH, T], MMDT, tag=f"aout{h}", name=f"aout{h}")
                for h in (1, 2)}
        aout01 = main.tile([128, T], MMDT, tag="aout01", name="aout01")
        accs = [main.tile([65, 512], F32, tag=f"acc{g}", name=f"acc{g}")
                for g in range(NG)]
        vsum = [[main.tile([128, 65], MMDT, tag=f"vs{h}{g}",
                           name=f"vs{h}{g}") for g in range(3)]
                for h in range(HPC)]

        # ============ phase 1: DMA in (xT pre-transposed on host) ========
        xT_ctx = ExitStack()
        xTp = xT_ctx.enter_context(tc.tile_pool(name="xTp", bufs=1))
        xT = [xTp.tile([128, T], MMDT, tag=f"xT{f}", name=f"xT{f}")
              for f in range(KT)]

        for k in range(KT):
            nc.scalar.dma_start(wqk[k][:], d["wqk"][k * 128:(k + 1) * 128, :])
        nc.scalar.dma_start(bqk[:], d["bqk"])
        nc.scalar.dma_start(ones[:], d["ones"])
        # first column-chunk of xT in fine 256-col pieces across queues
        for k in range(KT):
            for half in range(2):
                c0 = 256 * half
                nc.sync.dma_start(
                    xT[k][:, c0:c0 + 256],
                    d["xt"][k * 128:(k + 1) * 128, c0:c0 + 256])
        for c in range(1, NG):
            for k in range(KT):
                nc.sync.dma_start(
                    xT[k][:, c * 512:(c + 1) * 512],
                    d["xt"][k * 128:(k + 1) * 128, c * 512:(c + 1) * 512])
        for k in range(KT):
            nc.scalar.dma_start(wv[k][:], d["wv"][k * 128:(k + 1) * 128, :])
        nc.scalar.dma_start(bv[:], d["bv"])
        nc.scalar.dma_start(wo01[:], d["wo01"])
        nc.scalar.dma_start(wo2[:], d["wo2"])

        # vaug softmax-denominator ones columns
        va4 = vaug.rearrange("p (h k c) -> p h k c", h=HPC, c=65)
        for h in range(HPC):
            nc.vector.tensor_copy(
                va4[:, h, :, 64], ones[:, 0:1].broadcast_to([128, NK]))

        # ============ phase 2: warmup + projections ============
        vau3 = vaug.rearrange("p (h x) -> p h x", h=HPC)
        with tc.tile_pool(name="pps", bufs=2, space="PSUM") as pps:
            wu = pps.tile([128, 384], F32, tag="wu", name="wu", bufs=1)
            for i in range(12):
                nc.tensor.matmul(wu[:], wqk[0][:, 0:128], wqk[0][:],
                                 start=True, stop=True,
                                 skip_group_check=True)
            for n in range(NG):
                for g in range(3):
                    ps = pps.tile([128, 512], F32, tag="qk", name=f"qk{g}_{n}")
                    for k in range(KT):
                        nc.tensor.matmul(
                            ps[:], wqk[k][:, g * 128:(g + 1) * 128],
                            xT[k][:, n * 512:(n + 1) * 512],
                            start=(k == 0), stop=(k == KT - 1))
                    nc.vector.tensor_scalar_add(
                        qkt[g][:, n * 512:(n + 1) * 512], ps[:],
                        bqk[:, g:g + 1])
                for tt in range(4 * n, min(4 * n + 4, 12)):
                    ps = pps.tile([128, VN], F32, tag="v", name=f"v{tt}")
                    for k in range(KT):
                        nc.tensor.matmul(
                            ps[:], xT[k][:, tt * 128:(tt + 1) * 128], wv[k][:],
                            start=(k == 0), stop=(k == KT - 1))
                    nc.vector.tensor_add(
                        vau3[:, :, tt * 65:tt * 65 + 64],
                        ps.rearrange("p (h c) -> p h c", h=HPC),
                        bv.rearrange("p (h c) -> p h c", h=HPC))
            # alt2 = T3 with halves swapped (partition-shifting DMAs), so
            # h2's consecutive k-tiles can use alternating row groups
            nc.sync.dma_start(alt2[0:64, :], qkt[2][64:128, :])
            nc.sync.dma_start(alt2[64:128, :], qkt[2][0:64, :])

        def vsums():
            # masked-tile V sums: vsum[h][g] = sum_{ki >= 4(g+1)} vaug_ki,
            # consumed by one extra attn@v matmul against the ones tile
            with nc.allow_low_precision(reason="sum of <=12 fp16 values"):
                for h in range(HPC):
                    va3 = vaug[:, 1040 * h:1040 * (h + 1)].rearrange(
                        "p (k c) -> p c k", c=65)
                    for g in range(3):
                        nc.vector.tensor_reduce(
                            vsum[h][g][:], va3[:, :, 4 * (g + 1):NK],
                            axis=mybir.AxisListType.X, op=ALU.add)

        # ============ phase 3+4: attention pipeline + O-projection ======
        headqk = [
            (qkt[0][0:64, :], qkt[1][0:64, :]),      # h0: base 0
            (qkt[0][64:128, :], qkt[1][64:128, :]),  # h1: base 64
            (qkt[2][0:64, :], alt2[0:64, :]),        # h2: base 0
        ]
        fill1 = nc.gpsimd.to_reg(1.0)

        with tc.tile_pool(name="ep", bufs=12) as ep, \
             tc.tile_pool(name="fin", bufs=2) as fin, \
             tc.tile_pool(name="outp", bufs=3) as outp, \
             tc.tile_pool(name="sps", bufs=2, space="PSUM") as sps, \
             tc.tile_pool(name="ops", bufs=2, space="PSUM") as ops, \
             tc.tile_pool(name="oprj", bufs=2, space="PSUM") as oprj:

            erows = {}

            def unit_score_steps(u):
                h, J = u
                qT, kT = headqk[h]
                steps = []
                for j in range(4):
                    ki = 4 * J + j

                    def step(ki=ki):
                        lo = 128 * ki
                        e = ep.tile([128, T], MMDT, tag="e", name=f"e{h}_{ki}")
                        erows[(h, ki)] = e
                        for P in range(lo // 1024, 2):
                            clo = max(lo, 1024 * P)
                            ps = sps.tile([128, 1024], F32, tag="s",
                                          name=f"s{h}_{ki}_{P}")
                            for n in range(2):
                                s0 = 1024 * P + 512 * n
                                if s0 + 512 <= lo:
                                    continue
                                a0 = max(s0, lo)  # trim masked band columns
                                nc.tensor.matmul(
                                    ps[:, a0 - 1024 * P:512 * (n + 1)],
                                    kT[:, lo:lo + 128], qT[:, a0:s0 + 512])
                            nc.scalar.activation(
                                e[:, clo:1024 * (P + 1)],
                                ps[:, clo - 1024 * P:1024], AF.Exp,
                                scale=0.125)
                        w0 = 512 * J  # start of this unit's read window
                        if lo > w0:
                            # always-masked band: exp(0) = 1
                            nc.gpsimd.memset(e[:, w0:lo], 1.0)
                        nc.gpsimd.affine_select(
                            e[:, lo:lo + 128], e[:, lo:lo + 128],
                            pattern=[[1, 128]], compare_op=ALU.is_ge,
                            fill=fill1, base=0,
                            channel_multiplier=-1)
                    steps.append(step)
                return steps

            def unit_attnv_groups(u):
                h, J = u
                groups = []
                for g in range(J, NG):

                    def grp(g=g):
                        po = ops.tile([65, 512], F32, tag="o",
                                      name=f"o{h}{J}{g}")
                        has_virtual = (J == g and g < 3)
                        for j in range(4):
                            ki = 4 * J + j
                            nc.tensor.matmul(
                                po[:],
                                vaug[:, 1040 * h + ki * 65:
                                     1040 * h + ki * 65 + 65],
                                erows[(h, ki)][:, 512 * g:512 * (g + 1)],
                                start=(j == 0),
                                stop=(j == 3 and not has_virtual))
                        if has_virtual:
                            # masked k-tiles: weight-1 contribution of the
                            # precomputed V suffix sums
                            nc.tensor.matmul(po[:], vsum[h][g][:], ones[:],
                                             start=False, stop=True)
                        if J == 0:
                            nc.vector.tensor_copy(accs[g][:], po[:])
                        else:
                            nc.vector.tensor_add(accs[g][:], po[:], accs[g][:])
                        if J == g:
                            den = fin.tile([1, 512], F32, tag="den",
                                           name=f"den{h}{g}")
                            scr = fin.tile([1, 512], F32, tag="scr",
                                           name=f"scr{h}{g}")
                            rb = fin.tile([DH, 512], F32, tag="rb",
                                          name=f"rb{h}{g}")
                            nc.vector.tensor_copy(den[:], accs[g][64:65, :])
                            nc.vector.reciprocal_approx_fast(scr[:], den[:])
                            nc.gpsimd.partition_broadcast(rb[:], scr[:])
                            if h == 0:
                                dst = aout01[0:64, 512 * g:512 * (g + 1)]
                            else:
                                dst = aout[h][:, 512 * g:512 * (g + 1)]
                            nc.vector.tensor_mul(
                                dst, accs[g][0:64, :], rb[:])
                            if h == 1:
                                # stack h1 under h0 (partition-shift DMA)
                                nc.sync.dma_start(
                                    aout01[64:128, 512 * g:512 * (g + 1)],
                                    aout[1][:, 512 * g:512 * (g + 1)])
                            if h == 2:
                                oproj_group(g)
                    groups.append(grp)
                return groups

            def oproj_group(tg):
                for tt in range(4 * tg, 4 * tg + 4):
                    ot = outp.tile([128, D], MMDT, tag="ot", name=f"ot{tt}")
                    for (n0, w) in ((0, 512), (512, 256)):
                        ps = oprj.tile([128, 512], F32, tag="op",
                                       name=f"op{tt}_{n0}")
                        nc.tensor.matmul(
                            ps[:, 0:w],
                            aout01[:, tt * 128:(tt + 1) * 128],
                            wo01[:, n0:n0 + w], start=True, stop=False)
                        nc.tensor.matmul(
                            ps[:, 0:w],
                            aout[2][:, tt * 128:(tt + 1) * 128],
                            wo2[:, n0:n0 + w], start=False, stop=True)
                        nc.vector.tensor_copy(ot[:, n0:n0 + w], ps[:, 0:w])
                    r = slice(tt * 128, (tt + 1) * 128)
                    nc.sync.dma_start(d["y"][r, 0:384], ot[:, 0:384])
                    nc.sync.dma_start(d["y"][r, 384:768], ot[:, 384:768])

            def v_group_s(tt, tag):
                ps = sps.tile([128, VN], F32, tag="s", name=f"v{tt}")
                for k in range(KT):
                    nc.tensor.matmul(
                        ps[:], xT[k][:, tt * 128:(tt + 1) * 128], wv[k][:],
                        start=(k == 0), stop=(k == KT - 1))
                nc.vector.tensor_add(
                    vau3[:, :, tt * 65:tt * 65 + 64],
                    ps.rearrange("p (h c) -> p h c", h=HPC),
                    bv.rearrange("p (h c) -> p h c", h=HPC))

            units = [(h, J) for h in range(HPC) for J in range(4)]
            pending = [lambda tt=tt: v_group_s(tt, "s") for tt in range(12, 16)]
            pending.append(vsums)
            for u in units:
                steps = unit_score_steps(u)
                per = (len(pending) + len(steps) - 1) // max(len(steps), 1)
                gi = 0
                for st in steps:
                    st()
                    for _ in range(per):
                        if gi < len(pending):
                            pending[gi]()
                            gi += 1
                while gi < len(pending):
                    pending[gi]()
                    gi += 1
                pending = unit_attnv_groups(u)
            for grp in pending:
                grp()

        xT_ctx.close()


_NC_CACHE = None


def _get_nc():
    global _NC_CACHE
    if _NC_CACHE is None:
        _NC_CACHE = build_nc()
    return _NC_CACHE


def _make_in_maps(residual_stream, W_q, b_q, W_k, b_k, W_v, b_v, W_o, b_o):
    xts = [np.ascontiguousarray(residual_stream[b].T).astype(NPDT)
           for b in range(residual_stream.shape[0])]
    in_maps = []
    for c in range(N_CORES):
        b = c // 4
        hs = [3 * (c % 4) + i for i in range(HPC)]
        cs = [slice(64 * h, 64 * h + 64) for h in hs]
        wqk = np.concatenate(
            [W_q[:, cs[0]], W_q[:, cs[1]], W_k[:, cs[0]], W_k[:, cs[1]],
             W_q[:, cs[2]], W_k[:, cs[2]]], axis=1).astype(NPDT)
        bqk = np.concatenate(
            [b_q[cs[0]], b_q[cs[1]], b_k[cs[0]], b_k[cs[1]],
             b_q[cs[2]], b_k[cs[2]]]).astype(np.float32)
        bqk = np.ascontiguousarray(bqk.reshape(3, 128).T)
        wv = np.ascontiguousarray(
            np.concatenate([W_v[:, s] for s in cs], axis=1)).astype(NPDT)
        bv = np.zeros((1, VN), dtype=np.float32)
        bv[0, :] = np.concatenate([b_v[s] for s in cs])
        bv = np.ascontiguousarray(np.broadcast_to(bv, (128, VN)))
        m = {
            "xt": xts[b],
            "wqk": wqk,
            "bqk": bqk,
            "wv": wv,
            "bv": bv,
            "ones": np.ones((128, 512), dtype=NPDT),
        }
        m["wo01"] = np.ascontiguousarray(
            W_o[64 * hs[0]:64 * hs[0] + 128, :]).astype(NPDT)
        m["wo2"] = np.ascontiguousarray(
            W_o[64 * hs[2]:64 * hs[2] + 64, :]).astype(NPDT)
        in_maps.append(m)
    return in_maps


def kernel(residual_stream, W_q, b_q, W_k, b_k, W_v, b_v, W_o, b_o,
           _trace=False):
    residual_stream = np.asarray(residual_stream, dtype=np.float32)
    args = [np.asarray(a, dtype=np.float32)
            for a in (W_q, b_q, W_k, b_k, W_v, b_v, W_o, b_o)]
    W_q, b_q, W_k, b_k, W_v, b_v, W_o, b_o = args
    nc = _get_nc()
    in_maps = _make_in_maps(residual_stream, W_q, b_q, W_k, b_k, W_v, b_v,
                            W_o, b_o)
    res = run_bass_kernel_spmd(nc, in_maps, core_ids=list(range(N_CORES)),
                               trace=_trace)
    B = residual_stream.shape[0]
    out = np.zeros((B, T, D), dtype=np.float32)
    for c in range(N_CORES):
        out[c // 4] += res.results[c]["y"].astype(np.float32)
    out += b_o[None, None, :]
    if _trace:
        kernel._last_result = res
    return out


# revision 35
# speedup vs baseline: 1.0198x; 1.0100x over previous
"""Trainium2 Bass kernel for nn_Attention_49366354100559.

Multi-head attention: B=2, T=2048, D=768, H=12, Dh=64.
Reference zeroes the upper triangle of scores (not -inf) before softmax,
so masked positions contribute exp(0)=1 to the softmax — the attention
matrix is dense in attn@v.

Sharding: 8 cores = 2 batches x 4 core-groups; each core computes 3 heads
of one batch and produces a partial [2048, 768] output (pre-W_o-bias);
host sums the 4 partials per batch and adds b_o.

Per-core device program (fp16 matmul operands, 1 cycle/row PE):
  1. x^T is pre-transposed and cast to fp16 on the HOST and DMA'd in as
     6 [128, 2048] tiles (column-chunked so projections start early);
     a dozen warmup matmuls un-throttle the PE HAM clock gate during the
     first DMA wait.
  2. q^T,k^T feature-major with W stationary (column groups [q0|q1],
     [k0|k1],[q2|k2] so each head's q/k share a partition base; k2 is
     moved to base 0 with an SBUF-SBUF DMA). v token-major with x^T
     stationary, plus an appended ones column (v_aug) so attn@v also
     accumulates the softmax denominator for free.
  3. Attention as a software pipeline over units (head, k-quad):
     scores^T[k,q] = k @ q^T on live columns only, exp on ACT straight
     out of PSUM; the always-masked band is memset to 1.0 and the
     128-wide causal edge fixed with affine_select(fill=1.0); attn@v of
     the previous unit's exp rows runs between score bursts to keep the
     PE dense (HAM stays warm). Fully-masked k-tiles are replaced by
     per-quad v column-sum suffixes.
  4. Finalize per (head, q-group): fast reciprocal of the denominator
     row, partition-broadcast, scale -> attn_out^T.
  5. O-projection per token tile, interleaved with the last head; y is
     written fp16 (host sums partials in fp32).
"""

import sys

import numpy as np

if "/opt/trn_rl_repo" not in sys.path:
    sys.path.insert(0, "/opt/trn_rl_repo")

import concourse.mybir as mybir
from concourse import bacc
from concourse.tile import TileContext
from concourse.bass_utils import run_bass_kernel_spmd

F32 = mybir.dt.float32
F16 = mybir.dt.float16
AF = mybir.ActivationFunctionType
ALU = mybir.AluOpType

MMDT = F16
NPDT = np.float16

N_CORES = 8
VN = 192
T = 2048
D = 768
HPC = 3  # heads per core
DH = 64
NK = 16  # k-token tiles of 128
NG = 4  # q groups of 512
KT = 6  # contraction tiles for D=768


def build_nc():
    nc = bacc.Bacc("TRN2", target_bir_lowering=False, debug=False,
                   num_devices=N_CORES)
    d = {}
    d["xt"] = nc.dram_tensor("xt", [D, T], MMDT, kind="ExternalInput").ap()
    d["wqk"] = nc.dram_tensor("wqk", [D, 384], MMDT, kind="ExternalInput").ap()
    d["bqk"] = nc.dram_tensor("bqk", [128, 3], F32, kind="ExternalInput").ap()
    d["wv"] = nc.dram_tensor("wv", [D, VN], MMDT, kind="ExternalInput").ap()
    d["bv"] = nc.dram_tensor("bv", [128, VN], F32, kind="ExternalInput").ap()
    d["wo01"] = nc.dram_tensor("wo01", [128, D], MMDT,
                               kind="ExternalInput").ap()
    d["wo2"] = nc.dram_tensor("wo2", [DH, D], MMDT,
                              kind="ExternalInput").ap()
    d["ones"] = nc.dram_tensor("ones", [128, 512], MMDT,
                               kind="ExternalInput").ap()
    d["y"] = nc.dram_tensor("y", [T, D], MMDT, kind="ExternalOutput").ap()

    with TileContext(nc) as tc:
        _emit(nc, tc, d)
    nc.compile()
    return nc


def _emit(nc, tc, d):
    from contextlib import ExitStack

    with ExitStack() as ctx:
        wp = ctx.enter_context(tc.tile_pool(name="wp", bufs=1))
        main = ctx.enter_context(tc.tile_pool(name="main", bufs=1))

        # ---- weight/constant tiles ----
        wqk = [wp.tile([128, 384], MMDT, tag=f"wqk{k}", name=f"wqk{k}")
               for k in range(KT)]
        wv = [wp.tile([128, VN], MMDT, tag=f"wv{k}", name=f"wv{k}")
              for k in range(KT)]
        wo01 = wp.tile([128, D], MMDT, tag="wo01", name="wo01")
        wo2 = wp.tile([DH, D], MMDT, tag="wo2", name="wo2")
        bqk = wp.tile([128, 3], F32, tag="bqk", name="bqk")
        bv = wp.tile([128, VN], F32, tag="bv", name="bv")
        ones = wp.tile([128, 512], MMDT, tag="ones", name="ones")

        # ---- persistent SBUF ----
        qkt = [main.tile([128, T], MMDT, tag=f"qkt{g}", name=f"qkt{g}")
               for g in range(3)]  # [q0|q1], [k0|k1], [q2|k2]
        alt2 = main.tile([128, T], MMDT, tag="alt2", name="alt2")
        vaug = main.tile([128, HPC * NK * 65], MMDT, tag="vaug", name="vaug")
        aout = {h: main.tile([DH, T], MMDT, tag=f"aout{h}", name=f"aout{h}")
                for h in (1, 2)}
        aout01 = main.tile([128, T], MMDT, tag="aout01", name="aout01")
        accs = [main.tile([65, 512], F32, tag=f"acc{g}", name=f"acc{g}")
                for g in range(NG)]
        vsum = [[main.tile([128, 65], MMDT, tag=f"vs{h}{g}",
                           name=f"vs{h}{g}") for g in range(3)]
                for h in range(HPC)]

        # ============ phase 1: DMA in (xT pre-transposed on host) ========
        xT_ctx = ExitStack()
        xTp = xT_ctx.enter_context(tc.tile_pool(name="xTp", bufs=1))
        xT = [xTp.tile([128, T], MMDT, tag=f"xT{f}", name=f"xT{f}")
              for f in range(KT)]

        for k in range(KT):
            nc.scalar.dma_start(wqk[k][:], d["wqk"][k * 128:(k + 1) * 128, :])
        nc.scalar.dma_start(bqk[:], d["bqk"])
        nc.scalar.dma_start(ones[:], d["ones"])
        # first column-chunk of xT in fine 256-col pieces across queues
        for k in range(KT):
            for half in range(2):
                c0 = 256 * half
                nc.sync.dma_start(
                    xT[k][:, c0:c0 + 256],
                    d["xt"][k * 128:(k + 1) * 128, c0:c0 + 256])
        for k in range(3):
            nc.scalar.dma_start(wv[k][:], d["wv"][k * 128:(k + 1) * 128, :])
        for k in range(KT):
            nc.sync.dma_start(
                xT[k][:, 512:1024], d["xt"][k * 128:(k + 1) * 128, 512:1024])
        for k in range(3, KT):
            nc.scalar.dma_start(wv[k][:], d["wv"][k * 128:(k + 1) * 128, :])
        nc.scalar.dma_start(bv[:], d["bv"])
        for c in range(2, NG):
            for k in range(KT):
                nc.sync.dma_start(
                    xT[k][:, c * 512:(c + 1) * 512],
                    d["xt"][k * 128:(k + 1) * 128, c * 512:(c + 1) * 512])
        nc.scalar.dma_start(wo01[:], d["wo01"])
        nc.scalar.dma_start(wo2[:], d["wo2"])

        # vaug softmax-denominator ones columns
        va4 = vaug.rearrange("p (h k c) -> p h k c", h=HPC, c=65)
        for h in range(HPC):
            nc.vector.tensor_copy(
                va4[:, h, :, 64], ones[:, 0:1].broadcast_to([128, NK]))

        # ============ phase 2: warmup + projections ============
        vau3 = vaug.rearrange("p (h x) -> p h x", h=HPC)
        with tc.tile_pool(name="pps", bufs=2, space="PSUM") as pps:
            wu = pps.tile([128, 384], F32, tag="wu", name="wu", bufs=1)
            for i in range(10):
                nc.tensor.matmul(wu[:], wqk[0][:, 0:128], wqk[0][:],
                                 start=True, stop=True,
                                 skip_group_check=True)
            for n in range(NG):
                for g in range(3):
                    ps = pps.tile([128, 512], F32, tag="qk", name=f"qk{g}_{n}")
                    for k in range(KT):
                        nc.tensor.matmul(
                            ps[:], wqk[k][:, g * 128:(g + 1) * 128],
                            xT[k][:, n * 512:(n + 1) * 512],
                            start=(k == 0), stop=(k == KT - 1))
                    nc.vector.tensor_scalar_add(
                        qkt[g][:, n * 512:(n + 1) * 512], ps[:],
                        bqk[:, g:g + 1])
                for tt in range(4 * n, min(4 * n + 4, 12)):
                    ps = pps.tile([128, VN], F32, tag="v", name=f"v{tt}")
                    for k in range(KT):
                        nc.tensor.matmul(
                            ps[:], xT[k][:, tt * 128:(tt + 1) * 128], wv[k][:],
                            start=(k == 0), stop=(k == KT - 1))
                    nc.vector.tensor_add(
                        vau3[:, :, tt * 65:tt * 65 + 64],
                        ps.rearrange("p (h c) -> p h c", h=HPC),
                        bv.rearrange("p (h c) -> p h c", h=HPC))
            # alt2 = T3 with halves swapped (partition-shifting DMAs), so
            # h2's consecutive k-tiles can use alternating row groups
            nc.sync.dma_start(alt2[0:64, :], qkt[2][64:128, :])
            nc.sync.dma_start(alt2[64:128, :], qkt[2][0:64, :])

        def vsums():
            # masked-tile V sums: vsum[h][g] = sum_{ki >= 4(g+1)} vaug_ki,
            # consumed by one extra attn@v matmul against the ones tile
            with nc.allow_low_precision(reason="sum of <=12 fp16 values"):
                for h in range(HPC):
                    va3 = vaug[:, 1040 * h:1040 * (h + 1)].rearrange(
                        "p (k c) -> p c k", c=65)
                    for g in range(3):
                        nc.vector.tensor_reduce(
                            vsum[h][g][:], va3[:, :, 4 * (g + 1):NK],
                            axis=mybir.AxisListType.X, op=ALU.add)

        # ============ phase 3+4: attention pipeline + O-projection ======
        headqk = [
            (qkt[0][0:64, :], qkt[1][0:64, :]),      # h0: base 0
            (qkt[0][64:128, :], qkt[1][64:128, :]),  # h1: base 64
            (qkt[2][0:64, :], alt2[0:64, :]),        # h2: base 0
        ]
        fill1 = nc.gpsimd.to_reg(1.0)

        with tc.tile_pool(name="ep", bufs=12) as ep, \
             tc.tile_pool(name="fin", bufs=2) as fin, \
             tc.tile_pool(name="outp", bufs=3) as outp, \
             tc.tile_pool(name="sps", bufs=2, space="PSUM") as sps, \
             tc.tile_pool(name="ops", bufs=2, space="PSUM") as ops, \
             tc.tile_pool(name="oprj", bufs=2, space="PSUM") as oprj:

            erows = {}

            def unit_score_steps(u):
                h, J = u
                qT, kT = headqk[h]
                steps = []
                for j in range(4):
                    ki = 4 * J + j

                    def step(ki=ki):
                        lo = 128 * ki
                        e = ep.tile([128, T], MMDT, tag="e", name=f"e{h}_{ki}")
                        erows[(h, ki)] = e
                        for P in range(lo // 1024, 2):
                            clo = max(lo, 1024 * P)
                            ps = sps.tile([128, 1024], F32, tag="s",
                                          name=f"s{h}_{ki}_{P}")
                            for n in range(2):
                                s0 = 1024 * P + 512 * n
                                if s0 + 512 <= lo:
                                    continue
                                a0 = max(s0, lo)  # trim masked band columns
                                nc.tensor.matmul(
                                    ps[:, a0 - 1024 * P:512 * (n + 1)],
                                    kT[:, lo:lo + 128], qT[:, a0:s0 + 512])
                            nc.scalar.activation(
                                e[:, clo:1024 * (P + 1)],
                                ps[:, clo - 1024 * P:1024], AF.Exp,
                                scale=0.125)
                        w0 = 512 * J  # start of this unit's read window
                        if lo > w0:
                            # always-masked band: exp(0) = 1
                            nc.gpsimd.memset(e[:, w0:lo], 1.0)
                        nc.gpsimd.affine_select(
                            e[:, lo:lo + 128], e[:, lo:lo + 128],
                            pattern=[[1, 128]], compare_op=ALU.is_ge,
                            fill=fill1, base=0,
                            channel_multiplier=-1)
                    steps.append(step)
                return steps

            def unit_attnv_groups(u):
                h, J = u
                groups = []
                for g in range(J, NG):

                    def grp(g=g):
                        po = ops.tile([65, 512], F32, tag="o",
                                      name=f"o{h}{J}{g}")
                        has_virtual = (J == g and g < 3)
                        for j in range(4):
                            ki = 4 * J + j
                            nc.tensor.matmul(
                                po[:],
                                vaug[:, 1040 * h + ki * 65:
                                     1040 * h + ki * 65 + 65],
                                erows[(h, ki)][:, 512 * g:512 * (g + 1)],
                                start=(j == 0),
                                stop=(j == 3 and not has_virtual))
                        if has_virtual:
                            # masked k-tiles: weight-1 contribution of the
                            # precomputed V suffix sums
                            nc.tensor.matmul(po[:], vsum[h][g][:], ones[:],
                                             start=False, stop=True)
                        if J == 0:
                            nc.vector.tensor_copy(accs[g][:], po[:])
                        else:
                            nc.vector.tensor_add(accs[g][:], po[:], accs[g][:])
                        if J == g:
                            den = fin.tile([1, 512], F32, tag="den",
                                           name=f"den{h}{g}")
                            scr = fin.tile([1, 512], F32, tag="scr",
                                           name=f"scr{h}{g}")
                            rb = fin.tile([DH, 512], F32, tag="rb",
                                          name=f"rb{h}{g}")
                            nc.vector.tensor_copy(den[:], accs[g][64:65, :])
                            nc.vector.reciprocal_approx_fast(scr[:], den[:])
                            nc.gpsimd.partition_broadcast(rb[:], scr[:])
                            if h == 0:
                                dst = aout01[0:64, 512 * g:512 * (g + 1)]
                            else:
                                dst = aout[h][:, 512 * g:512 * (g + 1)]
                            nc.vector.tensor_mul(
                                dst, accs[g][0:64, :], rb[:])
                            if h == 1:
                                # stack h1 under h0 (partition-shift DMA)
                                nc.sync.dma_start(
                                    aout01[64:128, 512 * g:512 * (g + 1)],
                                    aout[1][:, 512 * g:512 * (g + 1)])
                            if h == 2:
                                oproj_group(g)
                    groups.append(grp)
                return groups

            def oproj_group(tg):
                for tt in range(4 * tg, 4 * tg + 4):
                    ot = outp.tile([128, D], MMDT, tag="ot", name=f"ot{tt}")
                    for (n0, w) in ((0, 512), (512, 256)):
                        ps = oprj.tile([128, 512], F32, tag="op",
                                       name=f"op{tt}_{n0}")
                        nc.tensor.matmul(
                            ps[:, 0:w],
                            aout01[:, tt * 128:(tt + 1) * 128],
                            wo01[:, n0:n0 + w], start=True, stop=False)
                        nc.tensor.matmul(
                            ps[:, 0:w],
                            aout[2][:, tt * 128:(tt + 1) * 128],
                            wo2[:, n0:n0 + w], start=False, stop=True)
                        nc.vector.tensor_copy(ot[:, n0:n0 + w], ps[:, 0:w])
                    r = slice(tt * 128, (tt + 1) * 128)
                    nc.sync.dma_start(d["y"][r, 0:384], ot[:, 0:384])
                    nc.sync.dma_start(d["y"][r, 384:768], ot[:, 384:768])

            def v_group_s(tt, tag):
                ps = sps.tile([128, VN], F32, tag="s", name=f"v{tt}")
                for k in range(KT):
                    nc.tensor.matmul(
                        ps[:], xT[k][:, tt * 128:(tt + 1) * 128], wv[k][:],
                        start=(k == 0), stop=(k == KT - 1))
                nc.vector.tensor_add(
                    vau3[:, :, tt * 65:tt * 65 + 64],
                    ps.rearrange("p (h c) -> p h c", h=HPC),
                    bv.rearrange("p (h c) -> p h c", h=HPC))

            units = [(h, J) for h in range(HPC) for J in range(4)]
            pending = [lambda tt=tt: v_group_s(tt, "s") for tt in range(12, 16)]
            pending.append(vsums)
            for u in units:
                steps = unit_score_steps(u)
                per = (len(pending) + len(steps) - 1) // max(len(steps), 1)
                gi = 0
                for st in steps:
                    st()
                    for _ in range(per):
                        if gi < len(pending):
                            pending[gi]()
                            gi += 1
                while gi < len(pending):
                    pending[gi]()
                    gi += 1
                pending = unit_attnv_groups(u)
            for grp in pending:
                grp()

        xT_ctx.close()


_NC_CACHE = None


def _get_nc():
    global _NC_CACHE
    if _NC_CACHE is None:
        _NC_CACHE = build_nc()
    return _NC_CACHE


def _make_in_maps(residual_stream, W_q, b_q, W_k, b_k, W_v, b_v, W_o, b_o):
    xts = [np.ascontiguousarray(residual_stream[b].T).astype(NPDT)
           for b in range(residual_stream.shape[0])]
    in_maps = []
    for c in range(N_CORES):
        b = c // 4
        hs = [3 * (c % 4) + i for i in range(HPC)]
        cs = [slice(64 * h, 64 * h + 64) for h in hs]
        wqk = np.concatenate(
            [W_q[:, cs[0]], W_q[:, cs[1]], W_k[:, cs[0]], W_k[:, cs[1]],
             W_q[:, cs[2]], W_k[:, cs[2]]], axis=1).astype(NPDT)
        bqk = np.concatenate(
            [b_q[cs[0]], b_q[cs[1]], b_k[cs[0]], b_k[cs[1]],
             b_q[cs[2]], b_k[cs[2]]]).astype(np.float32)
        bqk = np.ascontiguousarray(bqk.reshape(3, 128).T)
        wv = np.ascontiguousarray(
            np.concatenate([W_v[:, s] for s in cs], axis=1)).astype(NPDT)
        bv = np.zeros((1, VN), dtype=np.float32)
        bv[0, :] = np.concatenate([b_v[s] for s in cs])
        bv = np.ascontiguousarray(np.broadcast_to(bv, (128, VN)))
        m = {
            "xt": xts[b],
            "wqk": wqk,
            "bqk": bqk,
            "wv": wv,
            "bv": bv,
            "ones": np.ones((128, 512), dtype=NPDT),
        }
        m["wo01"] = np.ascontiguousarray(
            W_o[64 * hs[0]:64 * hs[0] + 128, :]).astype(NPDT)
        m["wo2"] = np.ascontiguousarray(
            W_o[64 * hs[2]:64 * hs[2] + 64, :]).astype(NPDT)
        in_maps.append(m)
    return in_maps


def kernel(residual_stream, W_q, b_q, W_k, b_k, W_v, b_v, W_o, b_o,
           _trace=False):
    residual_stream = np.asarray(residual_stream, dtype=np.float32)
    args = [np.asarray(a, dtype=np.float32)
            for a in (W_q, b_q, W_k, b_k, W_v, b_v, W_o, b_o)]
    W_q, b_q, W_k, b_k, W_v, b_v, W_o, b_o = args
    nc = _get_nc()
    in_maps = _make_in_maps(residual_stream, W_q, b_q, W_k, b_k, W_v, b_v,
                            W_o, b_o)
    res = run_bass_kernel_spmd(nc, in_maps, core_ids=list(range(N_CORES)),
                               trace=_trace)
    B = residual_stream.shape[0]
    out = np.zeros((B, T, D), dtype=np.float32)
    for c in range(N_CORES):
        out[c // 4] += res.results[c]["y"].astype(np.float32)
    out += b_o[None, None, :]
    if _trace:
        kernel._last_result = res
    return out


# revision 36
# speedup vs baseline: 1.0280x; 1.0080x over previous
"""Trainium2 Bass kernel for nn_Attention_49366354100559.

Multi-head attention: B=2, T=2048, D=768, H=12, Dh=64.
Reference zeroes the upper triangle of scores (not -inf) before softmax,
so masked positions contribute exp(0)=1 to the softmax — the attention
matrix is dense in attn@v.

Sharding: 8 cores = 2 batches x 4 core-groups; each core computes 3 heads
of one batch and produces a partial [2048, 768] output (pre-W_o-bias);
host sums the 4 partials per batch and adds b_o.

Per-core device program (fp16 matmul operands, 1 cycle/row PE):
  1. x^T is pre-transposed and cast to fp16 on the HOST and DMA'd in as
     6 [128, 2048] tiles (column-chunked so projections start early);
     a dozen warmup matmuls un-throttle the PE HAM clock gate during the
     first DMA wait.
  2. q^T,k^T feature-major with W stationary (column groups [q0|q1],
     [k0|k1],[q2|k2] so each head's q/k share a partition base; k2 is
     moved to base 0 with an SBUF-SBUF DMA). v token-major with x^T
     stationary, plus an appended ones column (v_aug) so attn@v also
     accumulates the softmax denominator for free.
  3. Attention as a software pipeline over units (head, k-quad):
     scores^T[k,q] = k @ q^T on live columns only, exp on ACT straight
     out of PSUM; the always-masked band is memset to 1.0 and the
     128-wide causal edge fixed with affine_select(fill=1.0); attn@v of
     the previous unit's exp rows runs between score bursts to keep the
     PE dense (HAM stays warm). Fully-masked k-tiles are replaced by
     per-quad v column-sum suffixes.
  4. Finalize per (head, q-group): fast reciprocal of the denominator
     row, partition-broadcast, scale -> attn_out^T.
  5. O-projection per token tile, interleaved with the last head; y is
     written fp16 (host sums partials in fp32).
"""

import sys

import numpy as np

if "/opt/trn_rl_repo" not in sys.path:
    sys.path.insert(0, "/opt/trn_rl_repo")

import concourse.mybir as mybir
from concourse import bacc
from concourse.tile import TileContext
from concourse.bass_utils import run_bass_kernel_spmd

F32 = mybir.dt.float32
F16 = mybir.dt.float16
AF = mybir.ActivationFunctionType
ALU = mybir.AluOpType

MMDT = F16
NPDT = np.float16

N_CORES = 8
VN = 192
T = 2048
D = 768
HPC = 3  # heads per core
DH = 64
NK = 16  # k-token tiles of 128
NG = 4  # q groups of 512
KT = 6  # contraction tiles for D=768


def build_nc():
    nc = bacc.Bacc("TRN2", target_bir_lowering=False, debug=False,
                   num_devices=N_CORES)
    d = {}
    d["xt"] = nc.dram_tensor("xt", [D, T], MMDT, kind="ExternalInput").ap()
    d["wqk"] = nc.dram_tensor("wqk", [D, 384], MMDT, kind="ExternalInput").ap()
    d["bqk"] = nc.dram_tensor("bqk", [128, 3], F32, kind="ExternalInput").ap()
    d["wv"] = nc.dram_tensor("wv", [D, VN], MMDT, kind="ExternalInput").ap()
    d["bv"] = nc.dram_tensor("bv", [128, VN], F32, kind="ExternalInput").ap()
    d["wo01"] = nc.dram_tensor("wo01", [128, D], MMDT,
                               kind="ExternalInput").ap()
    d["wo2"] = nc.dram_tensor("wo2", [DH, D], MMDT,
                              kind="ExternalInput").ap()
    d["ones"] = nc.dram_tensor("ones", [128, 512], MMDT,
                               kind="ExternalInput").ap()
    d["y"] = nc.dram_tensor("y", [T, D], MMDT, kind="ExternalOutput").ap()

    with TileContext(nc) as tc:
        _emit(nc, tc, d)
    nc.compile()
    return nc


def _emit(nc, tc, d):
    from contextlib import ExitStack

    with ExitStack() as ctx:
        wp = ctx.enter_context(tc.tile_pool(name="wp", bufs=1))
        main = ctx.enter_context(tc.tile_pool(name="main", bufs=1))

        # ---- weight/constant tiles ----
        wqk = [wp.tile([128, 384], MMDT, tag=f"wqk{k}", name=f"wqk{k}")
               for k in range(KT)]
        wv = [wp.tile([128, VN], MMDT, tag=f"wv{k}", name=f"wv{k}")
              for k in range(KT)]
        wo01 = wp.tile([128, D], MMDT, tag="wo01", name="wo01")
        wo2 = wp.tile([DH, D], MMDT, tag="wo2", name="wo2")
        bqk = wp.tile([128, 3], F32, tag="bqk", name="bqk")
        bv = wp.tile([128, VN], F32, tag="bv", name="bv")
        ones = wp.tile([128, 512], MMDT, tag="ones", name="ones")

        # ---- persistent SBUF ----
        qkt = [main.tile([128, T], MMDT, tag=f"qkt{g}", name=f"qkt{g}")
               for g in range(3)]  # [q0|q1], [k0|k1], [q2|k2]
        alt2 = main.tile([128, T], MMDT, tag="alt2", name="alt2")
        vaug = main.tile([128, HPC * NK * 65], MMDT, tag="vaug", name="vaug")
        aout = {h: main.tile([DH, T], MMDT, tag=f"aout{h}", name=f"aout{h}")
                for h in (1, 2)}
        aout01 = main.tile([128, T], MMDT, tag="aout01", name="aout01")
        accs = [main.tile([65, 512], F32, tag=f"acc{g}", name=f"acc{g}")
                for g in range(NG)]
        vsum = [[main.tile([128, 65], MMDT, tag=f"vs{h}{g}",
                           name=f"vs{h}{g}") for g in range(3)]
                for h in range(HPC)]

        # ============ phase 1: DMA in (xT pre-transposed on host) ========
        xT_ctx = ExitStack()
        xTp = xT_ctx.enter_context(tc.tile_pool(name="xTp", bufs=1))
        xT = [xTp.tile([128, T], MMDT, tag=f"xT{f}", name=f"xT{f}")
              for f in range(KT)]

        for k in range(KT):
            nc.scalar.dma_start(wqk[k][:], d["wqk"][k * 128:(k + 1) * 128, :])
        nc.scalar.dma_start(bqk[:], d["bqk"])
        nc.scalar.dma_start(ones[:], d["ones"])
        # first column-chunk of xT in fine 256-col pieces across queues
        for k in range(KT):
            for half in range(2):
                c0 = 256 * half
                nc.sync.dma_start(
                    xT[k][:, c0:c0 + 256],
                    d["xt"][k * 128:(k + 1) * 128, c0:c0 + 256])
        for c in range(1, NG):
            for k in range(KT):
                nc.sync.dma_start(
                    xT[k][:, c * 512:(c + 1) * 512],
                    d["xt"][k * 128:(k + 1) * 128, c * 512:(c + 1) * 512])
        for k in range(KT):
            nc.scalar.dma_start(wv[k][:], d["wv"][k * 128:(k + 1) * 128, :])
        nc.scalar.dma_start(bv[:], d["bv"])
        nc.scalar.dma_start(wo01[:], d["wo01"])
        nc.scalar.dma_start(wo2[:], d["wo2"])

        # vaug softmax-denominator ones columns
        va4 = vaug.rearrange("p (h k c) -> p h k c", h=HPC, c=65)
        for h in range(HPC):
            nc.vector.tensor_copy(
                va4[:, h, :, 64], ones[:, 0:1].broadcast_to([128, NK]))

        # ============ phase 2: warmup + projections ============
        vau3 = vaug.rearrange("p (h x) -> p h x", h=HPC)
        with tc.tile_pool(name="pps", bufs=2, space="PSUM") as pps:
            wu = pps.tile([128, 384], F32, tag="wu", name="wu", bufs=1)
            for i in range(12):
                nc.tensor.matmul(wu[:], wqk[0][:, 0:128], wqk[0][:],
                                 start=True, stop=True,
                                 skip_group_check=True)
            for n in range(NG):
                for g in range(3):
                    ps = pps.tile([128, 512], F32, tag="qk", name=f"qk{g}_{n}")
                    for k in range(KT):
                        nc.tensor.matmul(
                            ps[:], wqk[k][:, g * 128:(g + 1) * 128],
                            xT[k][:, n * 512:(n + 1) * 512],
                            start=(k == 0), stop=(k == KT - 1))
                    nc.vector.tensor_scalar_add(
                        qkt[g][:, n * 512:(n + 1) * 512], ps[:],
                        bqk[:, g:g + 1])
                for tt in range(4 * n, min(4 * n + 4, 12)):
                    ps = pps.tile([128, VN], F32, tag="v", name=f"v{tt}")
                    for k in range(KT):
                        nc.tensor.matmul(
                            ps[:], xT[k][:, tt * 128:(tt + 1) * 128], wv[k][:],
                            start=(k == 0), stop=(k == KT - 1))
                    nc.vector.tensor_add(
                        vau3[:, :, tt * 65:tt * 65 + 64],
                        ps.rearrange("p (h c) -> p h c", h=HPC),
                        bv.rearrange("p (h c) -> p h c", h=HPC))
            # alt2 = T3 with halves swapped (partition-shifting DMAs), so
            # h2's consecutive k-tiles can use alternating row groups
            nc.sync.dma_start(alt2[0:64, :], qkt[2][64:128, :])
            nc.sync.dma_start(alt2[64:128, :], qkt[2][0:64, :])

        def vsums():
            # masked-tile V sums: vsum[h][g] = sum_{ki >= 4(g+1)} vaug_ki,
            # consumed by one extra attn@v matmul against the ones tile
            with nc.allow_low_precision(reason="sum of <=12 fp16 values"):
                for h in range(HPC):
                    va3 = vaug[:, 1040 * h:1040 * (h + 1)].rearrange(
                        "p (k c) -> p c k", c=65)
                    for g in range(3):
                        nc.vector.tensor_reduce(
                            vsum[h][g][:], va3[:, :, 4 * (g + 1):NK],
                            axis=mybir.AxisListType.X, op=ALU.add)

        # ============ phase 3+4: attention pipeline + O-projection ======
        headqk = [
            (qkt[0][0:64, :], qkt[1][0:64, :]),      # h0: base 0
            (qkt[0][64:128, :], qkt[1][64:128, :]),  # h1: base 64
            (qkt[2][0:64, :], alt2[0:64, :]),        # h2: base 0
        ]
        fill1 = nc.gpsimd.to_reg(1.0)

        with tc.tile_pool(name="ep", bufs=12) as ep, \
             tc.tile_pool(name="fin", bufs=2) as fin, \
             tc.tile_pool(name="outp", bufs=3) as outp, \
             tc.tile_pool(name="sps", bufs=2, space="PSUM") as sps, \
             tc.tile_pool(name="ops", bufs=2, space="PSUM") as ops, \
             tc.tile_pool(name="oprj", bufs=2, space="PSUM") as oprj:

            erows = {}

            def unit_score_steps(u):
                h, J = u
                qT, kT = headqk[h]
                steps = []
                for j in range(4):
                    ki = 4 * J + j

                    def step(ki=ki):
                        lo = 128 * ki
                        e = ep.tile([128, T], MMDT, tag="e", name=f"e{h}_{ki}")
                        erows[(h, ki)] = e
                        for P in range(lo // 1024, 2):
                            clo = max(lo, 1024 * P)
                            ps = sps.tile([128, 1024], F32, tag="s",
                                          name=f"s{h}_{ki}_{P}")
                            for n in range(2):
                                s0 = 1024 * P + 512 * n
                                if s0 + 512 <= lo:
                                    continue
                                a0 = max(s0, lo)  # trim masked band columns
                                nc.tensor.matmul(
                                    ps[:, a0 - 1024 * P:512 * (n + 1)],
                                    kT[:, lo:lo + 128], qT[:, a0:s0 + 512])
                            nc.scalar.activation(
                                e[:, clo:1024 * (P + 1)],
                                ps[:, clo - 1024 * P:1024], AF.Exp,
                                scale=0.125)
                        w0 = 512 * J  # start of this unit's read window
                        if lo > w0:
                            # always-masked band: exp(0) = 1
                            nc.gpsimd.memset(e[:, w0:lo], 1.0)
                        nc.gpsimd.affine_select(
                            e[:, lo:lo + 128], e[:, lo:lo + 128],
                            pattern=[[1, 128]], compare_op=ALU.is_ge,
                            fill=fill1, base=0,
                            channel_multiplier=-1)
                    steps.append(step)
                return steps

            def unit_attnv_groups(u):
                h, J = u
                groups = []
                for g in range(J, NG):

                    def grp(g=g):
                        po = ops.tile([65, 512], F32, tag="o",
                                      name=f"o{h}{J}{g}")
                        has_virtual = (J == g and g < 3)
                        for j in range(4):
                            ki = 4 * J + j
                            nc.tensor.matmul(
                                po[:],
                                vaug[:, 1040 * h + ki * 65:
                                     1040 * h + ki * 65 + 65],
                                erows[(h, ki)][:, 512 * g:512 * (g + 1)],
                                start=(j == 0),
                                stop=(j == 3 and not has_virtual))
                        if has_virtual:
                            # masked k-tiles: weight-1 contribution of the
                            # precomputed V suffix sums
                            nc.tensor.matmul(po[:], vsum[h][g][:], ones[:],
                                             start=False, stop=True)
                        if J == 0:
                            nc.vector.tensor_copy(accs[g][:], po[:])
                        else:
                            nc.vector.tensor_add(accs[g][:], po[:], accs[g][:])
                        if J == g:
                            den = fin.tile([1, 512], F32, tag="den",
                                           name=f"den{h}{g}")
                            scr = fin.tile([1, 512], F32, tag="scr",
                                           name=f"scr{h}{g}")
                            rb = fin.tile([DH, 512], F32, tag="rb",
                                          name=f"rb{h}{g}")
                            nc.vector.tensor_copy(den[:], accs[g][64:65, :])
                            nc.vector.reciprocal_approx_fast(scr[:], den[:])
                            nc.gpsimd.partition_broadcast(rb[:], scr[:])
                            if h == 0:
                                dst = aout01[0:64, 512 * g:512 * (g + 1)]
                            else:
                                dst = aout[h][:, 512 * g:512 * (g + 1)]
                            nc.vector.tensor_mul(
                                dst, accs[g][0:64, :], rb[:])
                            if h == 1:
                                # stack h1 under h0 (partition-shift DMA)
                                nc.sync.dma_start(
                                    aout01[64:128, 512 * g:512 * (g + 1)],
                                    aout[1][:, 512 * g:512 * (g + 1)])
                            if h == 2:
                                oproj_group(g)
                    groups.append(grp)
                return groups

            def oproj_group(tg):
                for tt in range(4 * tg, 4 * tg + 4):
                    ot = outp.tile([128, D], MMDT, tag="ot", name=f"ot{tt}")
                    for (n0, w) in ((0, 512), (512, 256)):
                        ps = oprj.tile([128, 512], F32, tag="op",
                                       name=f"op{tt}_{n0}")
                        nc.tensor.matmul(
                            ps[:, 0:w],
                            aout01[:, tt * 128:(tt + 1) * 128],
                            wo01[:, n0:n0 + w], start=True, stop=False)
                        nc.tensor.matmul(
                            ps[:, 0:w],
                            aout[2][:, tt * 128:(tt + 1) * 128],
                            wo2[:, n0:n0 + w], start=False, stop=True)
                        nc.vector.tensor_copy(ot[:, n0:n0 + w], ps[:, 0:w])
                    r = slice(tt * 128, (tt + 1) * 128)
                    nc.sync.dma_start(d["y"][r, 0:384], ot[:, 0:384])
                    nc.sync.dma_start(d["y"][r, 384:768], ot[:, 384:768])

            def v_group_s(tt, tag):
                ps = sps.tile([128, VN], F32, tag="s", name=f"v{tt}")
                for k in range(KT):
                    nc.tensor.matmul(
                        ps[:], xT[k][:, tt * 128:(tt + 1) * 128], wv[k][:],
                        start=(k == 0), stop=(k == KT - 1))
                nc.vector.tensor_add(
                    vau3[:, :, tt * 65:tt * 65 + 64],
                    ps.rearrange("p (h c) -> p h c", h=HPC),
                    bv.rearrange("p (h c) -> p h c", h=HPC))

            units = [(h, J) for h in range(HPC) for J in range(4)]
            pending = [lambda tt=tt: v_group_s(tt, "s") for tt in range(12, 16)]
            pending.append(vsums)
            for u in units:
                steps = unit_score_steps(u)
                per = (len(pending) + len(steps) - 1) // max(len(steps), 1)
                gi = 0
                for st in steps:
                    st()
                    for _ in range(per):
                        if gi < len(pending):
                            pending[gi]()
                            gi += 1
                while gi < len(pending):
                    pending[gi]()
                    gi += 1
                pending = unit_attnv_groups(u)
            for grp in pending:
                grp()

        xT_ctx.close()


_NC_CACHE = None


def _get_nc():
    global _NC_CACHE
    if _NC_CACHE is None:
        _NC_CACHE = build_nc()
    return _NC_CACHE


def _make_in_maps(residual_stream, W_q, b_q, W_k, b_k, W_v, b_v, W_o, b_o):
    xts = [np.ascontiguousarray(residual_stream[b].T).astype(NPDT)
           for b in range(residual_stream.shape[0])]
    in_maps = []
    for c in range(N_CORES):
        b = c // 4
        hs = [3 * (c % 4) + i for i in range(HPC)]
        cs = [slice(64 * h, 64 * h + 64) for h in hs]
        wqk = np.concatenate(
            [W_q[:, cs[0]], W_q[:, cs[1]], W_k[:, cs[0]], W_k[:, cs[1]],
             W_q[:, cs[2]], W_k[:, cs[2]]], axis=1).astype(NPDT)
        bqk = np.concatenate(
            [b_q[cs[0]], b_q[cs[1]], b_k[cs[0]], b_k[cs[1]],
             b_q[cs[2]], b_k[cs[2]]]).astype(np.float32)
        bqk = np.ascontiguousarray(bqk.reshape(3, 128).T)
        wv = np.ascontiguousarray(
            np.concatenate([W_v[:, s] for s in cs], axis=1)).astype(NPDT)
        bv = np.zeros((1, VN), dtype=np.float32)
        bv[0, :] = np.concatenate([b_v[s] for s in cs])
        bv = np.ascontiguousarray(np.broadcast_to(bv, (128, VN)))
        m = {
            "xt": xts[b],
            "wqk": wqk,
            "bqk": bqk,
            "wv": wv,
            "bv": bv,
            "ones": np.ones((128, 512), dtype=NPDT),
        }
        m["wo01"] = np.ascontiguousarray(
            W_o[64 * hs[0]:64 * hs[0] + 128, :]).astype(NPDT)
        m["wo2"] = np.ascontiguousarray(
            W_o[64 * hs[2]:64 * hs[2] + 64, :]).astype(NPDT)
        in_maps.append(m)
    return in_maps


def kernel(residual_stream, W_q, b_q, W_k, b_k, W_v, b_v, W_o, b_o,
           _trace=False):
    residual_stream = np.asarray(residual_stream, dtype=np.float32)
    args = [np.asarray(a, dtype=np.float32)
            for a in (W_q, b_q, W_k, b_k, W_v, b_v, W_o, b_o)]
    W_q, b_q, W_k, b_k, W_v, b_v, W_o, b_o = args
    nc = _get_nc()
    in_maps = _make_in_maps(residual_stream, W_q, b_q, W_k, b_k, W_v, b_v,
                            W_o, b_o)
    res = run_bass_kernel_spmd(nc, in_maps, core_ids=list(range(N_CORES)),
                               trace=_trace)
    B = residual_stream.shape[0]
    out = np.zeros((B, T, D), dtype=np.float32)
    for c in range(N_CORES):
        out[c // 4] += res.results[c]["y"].astype(np.float32)
    out += b_o[None, None, :]
    if _trace:
        kernel._last_result = res
    return out
